# revision 30
# baseline (speedup 1.0000x reference)
"""Trainium2 Bass kernel for ContextualAttentionModule.

Data-parallel over batch: 8 samples -> 8 NeuronCores, one sample per core.
Per-core pipeline (C=256, H=W=32, L=1024 patches), v3:
  scores  = <fg_patch(p), bg_patch(l)> fp16 matmuls; +eps*G folded into
            psum eviction; /norm folded into exp scale
  prop    = 3x3 window-sum (separable DVE adds, f32)
  attn    = softmax over l (denominator via ones/rn column matmuls)
  recov   = conv_transpose(attn, kernels) via fp16 PE-transposed bank
  final   = recov*mask/9 + fg*(1-mask)
  out     = concat_g relu(dilated_conv_r(final) + b)  (fp16 matmuls)

Host ships small per-sample rows (mask/9, 1-mask, eps*G, 1/norm) so the
device spends no time on the scalar-row chains; all O(C*L*9) work (scores,
softmax, tconv, dilated convs) runs on device.

Engine rules honored: compute engines are lane-locked (partition i in ->
partition i out; SBUF operands of one op share a partition base; bases are
multiples of 32). Only PE, DMA and gpsimd partition_broadcast cross
partitions. GPSIMD cannot access PSUM.
"""

import numpy as np

import concourse.bass as bass
import concourse.tile as tile
from concourse import bacc, mybir
from concourse.bass_utils import run_bass_kernel_spmd
from concourse.masks import make_identity

F32 = mybir.dt.float32
F32R = mybir.dt.float32r
BF16 = mybir.dt.bfloat16
FP16 = mybir.dt.float16
U16 = mybir.dt.uint16
AF = mybir.ActivationFunctionType
ALU = mybir.AluOpType

EPS = 1e-7
RATES = (1, 2, 4, 8)
OFFS = [(dy, dx) for dy in range(3) for dx in range(3)]

_CACHE = {}


def build_program(debug=False):
    nc = bacc.Bacc()
    fg_d = nc.declare_dram_parameter("fg16", [256, 34, 34], FP16, isOutput=False)
    bg_d = nc.declare_dram_parameter("bgm16", [256, 34, 34], FP16, isOutput=False)
    m_d = nc.declare_dram_parameter("aux", [4, 1024], F32, isOutput=False)
    w_d = nc.declare_dram_parameter("wconv", [2, 128, 2304], FP16, isOutput=False)
    bgt_d = nc.declare_dram_parameter("bgt", [8, 128, 2304], FP16, isOutput=False)
    b_d = nc.declare_dram_parameter("bias", [256, 1], F32, isOutput=False)
    out_d = nc.declare_dram_parameter("out", [256, 32, 32], F32, isOutput=True)
    dbg = {}
    if debug:
        for nm, shp in [("d_drow", [1, 1024]), ("d_boxs2", [1, 1024]),
                        ("d_final", [128, 32, 32])]:
            dbg[nm] = nc.declare_dram_parameter(nm, shp, F32, isOutput=True)

    with tile.TileContext(nc) as tc:
        _emit(nc, tc, fg_d, bg_d, m_d, w_d, b_d, bgt_d, out_d, dbg)
    nc.compile()
    return nc


def _ring_zero16(nc, buf, eng, n=34):
    eng.memset(buf[:, 0:n:n - 1, :].bitcast(U16), 0)
    eng.memset(buf[:, 1:n - 1, 0:n:n - 1].bitcast(U16), 0)


def _ring_zero32(nc, buf, eng, n=34):
    eng.memset(buf[:, 0:n:n - 1, :].bitcast(F32), 0.0)
    eng.memset(buf[:, 1:n - 1, 0:n:n - 1].bitcast(F32), 0.0)


def _boxsum(nc, scr, src_pad, dst, eng):
    """3x3 SAME window sum on [p,34,34] ring-zeroed tiles -> [p,32,32]."""
    eng.tensor_tensor(scr[:, 1:33, 1:33], src_pad[:, 1:33, 0:32],
                      src_pad[:, 1:33, 1:33], ALU.add)
    eng.tensor_tensor(scr[:, 1:33, 1:33], scr[:, 1:33, 1:33],
                      src_pad[:, 1:33, 2:34], ALU.add)
    eng.tensor_tensor(dst[:], scr[:, 0:32, 1:33], scr[:, 1:33, 1:33], ALU.add)
    eng.tensor_tensor(dst[:], dst[:], scr[:, 2:34, 1:33], ALU.add)


def _emit(nc, tc, fg_d, bg_d, m_d, w_d, b_d, bgt_d, out_d, dbg=None):
    dbg = dbg or {}
    with tc.tile_pool(name="main", bufs=1) as main:
        # ----- persistent tiles -----
        fg_pad = [main.tile([128, 34, 34], FP16, name=f"fg_pad{c}") for c in range(2)]
        bgs = [[main.tile([128, 32, 32], FP16, name=f"bgs{c}_{d}") for d in range(9)]
               for c in range(2)]
        E = [main.tile([128, 34, 34], BF16, name=f"E{t}") for t in range(8)]
        bgT = [main.tile([128, 2304], FP16, name=f"bgT{t}") for t in range(8)]
        S = [main.tile([128, 34, 34], F32R, name=f"S{i}") for i in range(2)]
        WS = [main.tile([128, 34, 34], F32R, name=f"WS{i}") for i in range(2)]
        HS = [main.tile([128, 32, 32], F32R, name=f"HS{i}") for i in range(2)]
        maskb9 = main.tile([128, 32, 32], F32, name="maskb9")
        invmaskb = main.tile([128, 32, 32], F32, name="invmaskb")
        epsboxgB = main.tile([128, 32, 32], F32R, name="epsboxgB")
        Db = main.tile([128, 32, 32], F32R, name="Db")
        rncol = main.tile([128, 8], F32, name="rncol")
        W2 = main.tile([128, 16], BF16, name="W2")
        onesrow16 = main.tile([1, 128], FP16, name="onesrow16")
        wsb = [main.tile([128, 2304], FP16, name=f"wsb{c}") for c in range(2)]
        biasb = [main.tile([128, 1], F32, name=f"biasb{c}") for c in range(2)]
        fscr2 = [main.tile([128, 32, 32], FP16, name=f"fscr2_{c}") for c in range(2)]
        # Row scratch at partition 0 (s2 chain only).
        chainpad = main.tile([1, 34, 68], F32R, name="chainpad")
        cpad = chainpad[:, :, 0:34]
        cscr = chainpad[:, :, 34:68]
        boxA = main.tile([1, 32, 32], F32R, name="boxA")
        rowX = main.tile([1, 1024], F32R, name="rowX")   # rdrow (1/D)
        rowY = main.tile([1, 1024], F32, name="rowY")    # mask/9 row
        rowZ = main.tile([1, 1024], F32, name="rowZ")    # 1-mask row -> s2row
        rowG = main.tile([1, 1024], F32R, name="rowG")   # eps*G row
        s2stage = main.tile([33, 1024], F32, name="s2stage")
        boxs2_16 = main.tile([1, 32, 32], FP16, name="boxs2_16")

        with tc.tile_pool(name="stage", bufs=1) as stage:
            bg_pad = [stage.tile([128, 34, 34], FP16, name=f"bg_pad{c}")
                      for c in range(2)]

            # ----- DMAs (issue cost: Pool 25ns, SP 565ns, Act 667ns) -----
            nc.gpsimd.dma_start(rowY[0:1, :], m_d[0:1])
            nc.gpsimd.dma_start(rowZ[0:1, :], m_d[1:2])
            nc.gpsimd.dma_start(rowG[0:1, :].bitcast(F32), m_d[2:3])
            nc.sync.dma_start(bg_pad[0][:], bg_d[0:128])
            nc.sync.dma_start(bg_pad[1][:], bg_d[128:256])
            nc.scalar.dma_start(fg_pad[0][:], fg_d[0:128])
            nc.scalar.dma_start(fg_pad[1][:], fg_d[128:256])
            nc.sync.dma_start(rncol[:, :],
                              m_d[3:4].rearrange("o (p u) -> (o p) u", u=8))
            nc.sync.dma_start(wsb[0][:], w_d[0])
            nc.sync.dma_start(wsb[1][:], w_d[1])
            nc.sync.dma_start(biasb[0][:], b_d[0:128])
            nc.sync.dma_start(biasb[1][:], b_d[128:256])
            for t in range(8):
                (nc.sync if t % 2 == 0 else nc.scalar).dma_start(
                    bgT[t][:], bgt_d[t])

            # ----- Pool constants -----
            nc.gpsimd.memset(onesrow16[:], 1.0)
            nc.gpsimd.memset(W2[:], 1.0)

            # bgs shifts in psc consumption order:
            # DVE d{0,3,6}+c0d8, Act d{1,4,7}+c1d8, Pool d{2,5}
            for d in (0, 3, 6, 8):
                dy, dx = OFFS[d]
                nc.vector.tensor_copy(bgs[0][d][:],
                                      bg_pad[0][:, dy:dy + 32, dx:dx + 32])
            for d in (0, 3, 6, 8):
                dy, dx = OFFS[d]
                nc.vector.tensor_copy(bgs[1][d][:],
                                      bg_pad[1][:, dy:dy + 32, dx:dx + 32])
            for d in (1, 4, 7):
                dy, dx = OFFS[d]
                nc.scalar.copy(bgs[0][d][:],
                               bg_pad[0][:, dy:dy + 32, dx:dx + 32])
            for d in (1, 4, 7):
                dy, dx = OFFS[d]
                nc.scalar.copy(bgs[1][d][:],
                               bg_pad[1][:, dy:dy + 32, dx:dx + 32])
            for c in range(2):
                for d in (2, 5):
                    dy, dx = OFFS[d]
                    nc.gpsimd.tensor_copy(bgs[c][d][:],
                                          bg_pad[c][:, dy:dy + 32, dx:dx + 32])
            nc.gpsimd.partition_broadcast(
                epsboxgB.rearrange("p a b -> p (a b)"), rowG[0:1, :])
            nc.gpsimd.partition_broadcast(
                invmaskb.rearrange("p a b -> p (a b)"), rowZ[0:1, :])
            nc.gpsimd.partition_broadcast(
                maskb9.rearrange("p a b -> p (a b)"), rowY[0:1, :])
            _ring_zero32(nc, cpad, nc.vector)
            _ring_zero32(nc, cscr, nc.vector)
            for i in range(2):
                _ring_zero32(nc, S[i], nc.vector)
                _ring_zero32(nc, WS[i], nc.vector)
            for t in range(8):
                _ring_zero16(nc, E[t], nc.gpsimd)
            # fg*(1-mask), staged for phase 7 (Pool has slack here)
            for c in range(2):
                nc.gpsimd.tensor_tensor(fscr2[c][:], fg_pad[c][:, 1:33, 1:33],
                                        invmaskb[:], ALU.mult)

            # ================= scores phase =================
            with (
                tc.tile_pool(name="ps_sc", bufs=4, space="PSUM") as ps_sc,
                tc.tile_pool(name="ps_d", bufs=2, space="PSUM") as ps_d,
            ):
                blocks = [(c, d) for c in range(2) for d in range(9)]

                def psc_mms(t, ch):
                    psc = ps_sc.tile([128, 512], F32, name="psc", tag="psc")
                    r0 = 16 * ch
                    i = 0
                    for c in range(2):
                        for d, (dy, dx) in enumerate(OFFS):
                            nc.tensor.matmul(
                                psc[:],
                                bgs[c][d].rearrange("p a b -> p (a b)")
                                [:, 128 * t:128 * (t + 1)],
                                fg_pad[c][:, r0 + dy:r0 + dy + 16, dx:dx + 32],
                                start=(i == 0), stop=(i == 17))
                            i += 1
                    return psc

                # psd[ch]: partition 0 = D = sum_l E; partition 32 = sum_l rn*E
                psd = [ps_d.tile([33, 512], F32, name=f"psd{ch}", tag="prd")
                       for ch in range(2)]

                def psd_mm(u, chs=(0, 1)):
                    for ch in chs:
                        r0 = 16 * ch
                        mv = E[u][:, 1 + r0:17 + r0, 1:33]
                        nc.tensor.matmul(psd[ch][0:1, :], W2[:, 2 * u:2 * u + 1],
                                         mv, start=(u == 0), stop=(u == 7))
                        nc.tensor.matmul(psd[ch][32:33, :],
                                         W2[:, 2 * u + 1:2 * u + 2],
                                         mv, start=(u == 0), stop=(u == 7))

                def boxexp(t):
                    w, h = WS[t % 2], HS[t % 2]
                    sp = S[t % 2]
                    nc.vector.tensor_tensor(w[:, 1:33, 1:33], sp[:, 1:33, 0:32],
                                            sp[:, 1:33, 1:33], ALU.add)
                    nc.vector.tensor_tensor(w[:, 1:33, 1:33], w[:, 1:33, 1:33],
                                            sp[:, 1:33, 2:34], ALU.add)
                    nc.vector.tensor_tensor(h[:], w[:, 0:32, 1:33],
                                            w[:, 1:33, 1:33], ALU.add)
                    nc.vector.tensor_tensor(h[:], h[:], w[:, 2:34, 1:33], ALU.add)
                    nc.scalar.activation(E[t][:, 1:33, 1:33], h[:], AF.Exp,
                                         scale=rncol[:, t:t + 1])

                def boxexp_split(t):
                    # final slot: split rows DVE/Pool and pipeline exp halves
                    # with the closing denominator matmuls
                    w, h = WS[t % 2], HS[t % 2]
                    sp = S[t % 2]
                    nc.vector.tensor_tensor(w[:, 1:18, 1:33], sp[:, 1:18, 0:32],
                                            sp[:, 1:18, 1:33], ALU.add)
                    nc.vector.tensor_tensor(w[:, 1:18, 1:33], w[:, 1:18, 1:33],
                                            sp[:, 1:18, 2:34], ALU.add)
                    nc.gpsimd.tensor_tensor(w[:, 18:33, 1:33], sp[:, 18:33, 0:32],
                                            sp[:, 18:33, 1:33], ALU.add)
                    nc.gpsimd.tensor_tensor(w[:, 18:33, 1:33], w[:, 18:33, 1:33],
                                            sp[:, 18:33, 2:34], ALU.add)
                    nc.vector.tensor_tensor(h[:, 0:16, :], w[:, 0:16, 1:33],
                                            w[:, 1:17, 1:33], ALU.add)
                    nc.vector.tensor_tensor(h[:, 0:16, :], h[:, 0:16, :],
                                            w[:, 2:18, 1:33], ALU.add)
                    nc.scalar.activation(E[t][:, 1:17, 1:33], h[:, 0:16, :],
                                         AF.Exp, scale=rncol[:, t:t + 1])
                    psd_mm(t, chs=(0,))
                    nc.vector.tensor_tensor(h[:, 16:32, :], w[:, 16:32, 1:33],
                                            w[:, 17:33, 1:33], ALU.add)
                    nc.vector.tensor_tensor(h[:, 16:32, :], h[:, 16:32, :],
                                            w[:, 18:34, 1:33], ALU.add)
                    nc.scalar.activation(E[t][:, 17:33, 1:33], h[:, 16:32, :],
                                         AF.Exp, scale=rncol[:, t:t + 1])
                    psd_mm(t, chs=(1,))

                def evict(t, ch, psc):
                    r0 = 16 * ch
                    nc.vector.tensor_tensor(
                        S[t % 2][:, 1 + r0:17 + r0, 1:33], psc[:],
                        epsboxgB[:, r0:16 + r0, :], ALU.add)

                # ---- slots 0..6 ----
                for t in range(7):
                    for ch in range(2):
                        psc = psc_mms(t, ch)
                        evict(t, ch, psc)
                    boxexp(t)
                    if t == 0:
                        # W2 = [1, rn_0, 1, rn_1, ...] (bf16)
                        nc.vector.tensor_copy(W2[:, 1:16:2], rncol[:, 0:8])
                    # PE tail of slot: lagged denominator matmuls
                    if t >= 2:
                        psd_mm(t - 2)
                # ---- slot 7: close out denominators while E7 resolves ----
                for ch in range(2):
                    psc = psc_mms(7, ch)
                    evict(7, ch, psc)
                psd_mm(5)
                psd_mm(6)
                # boxexp_split emits psd(7) halves interleaved with exp
                boxexp_split(7)

                # denominator -> reciprocal -> broadcast -> divide; E[0]
                # is divided per 16-row half so tconv can start earlier
                for ch in range(2):
                    nc.vector.reciprocal(
                        rowX[0:1, 512 * ch:512 * (ch + 1)].bitcast(F32),
                        psd[ch][0:1, :])
                    nc.gpsimd.partition_broadcast(
                        Db[:, 16 * ch:16 * (ch + 1), :]
                        .rearrange("p a b -> p (a b)"),
                        rowX[0:1, 512 * ch:512 * (ch + 1)])
                    r0 = 16 * ch
                    nc.vector.tensor_tensor(E[0][:, 1 + r0:17 + r0, 1:33],
                                            E[0][:, 1 + r0:17 + r0, 1:33],
                                            Db[:, r0:16 + r0, :], ALU.mult)
                for t in range(1, 8):
                    if t < 5:
                        nc.vector.tensor_tensor(E[t][:, 1:33, 1:33],
                                                E[t][:, 1:33, 1:33], Db[:],
                                                ALU.mult)
                    else:
                        nc.gpsimd.tensor_tensor(E[t][:, 1:33, 1:33],
                                                E[t][:, 1:33, 1:33], Db[:],
                                                ALU.mult)
                # s2 = eps * (sum_l rn*E) / D; lane-aligned psum escape via
                # Act copy (32->32) then cross-partition DMA (32->0)
                for ch in range(2):
                    nc.scalar.copy(s2stage[32:33, 512 * ch:512 * (ch + 1)],
                                   psd[ch][32:33, :])
                nc.sync.dma_start(rowZ[0:1, 0:512], s2stage[32:33, 0:512])
                nc.gpsimd.dma_start(rowZ[0:1, 512:1024],
                                    s2stage[32:33, 512:1024])
                for ch in range(2):
                    r0 = 16 * ch
                    nc.vector.scalar_tensor_tensor(
                        out=cpad[:, 1 + r0:17 + r0, 1:33],
                        in0=rowZ[0:1, 512 * ch:512 * (ch + 1)], scalar=EPS,
                        in1=rowX[0:1, 512 * ch:512 * (ch + 1)],
                        op0=ALU.mult, op1=ALU.mult)
                _boxsum(nc, cscr, cpad, boxA, nc.vector)
                nc.vector.tensor_copy(boxs2_16[:], boxA)
                if dbg:
                    nc.sync.dma_start(dbg["d_drow"][:], rowX[0:1, :].bitcast(F32))
                    nc.sync.dma_start(
                        dbg["d_boxs2"][:],
                        boxA.rearrange("o a b -> o (a b)").bitcast(F32))
        # ----- stage + scores psum pools closed -----

        with tc.tile_pool(name="late", bufs=1) as late:
            final_pad = [late.tile([128, 48, 48], FP16, name=f"final_pad{c}")
                         for c in range(2)]
            fscr = [late.tile([128, 32, 32], F32, name=f"fscr{c}") for c in range(2)]
            for c in range(2):
                nc.gpsimd.memset(final_pad[c][:, 0:8, :].bitcast(U16), 0)
                nc.gpsimd.memset(final_pad[c][:, 40:48, :].bitcast(U16), 0)
                nc.gpsimd.memset(final_pad[c][:, 8:40, 0:8].bitcast(U16), 0)
                nc.gpsimd.memset(final_pad[c][:, 8:40, 40:48].bitcast(U16), 0)

            with (
                tc.tile_pool(name="ps_rec", bufs=1, space="PSUM") as ps_rec,
                tc.tile_pool(name="ps_o", bufs=3, space="PSUM") as ps_o,
            ):
                prec = [[ps_rec.tile([128, 512], F32, name=f"prec{c}_{ch}")
                         for ch in range(2)] for c in range(2)]
                # ---- tconv: contraction over (l, d), c-outer ----
                d_first = [6, 7, 8, 3, 4, 5, 0, 1, 2]  # dy=2 rows first
                for t in range(8):
                    for c in range(2):
                        for ch in range(2):
                            dlist = d_first if (c == 0 and t == 0) \
                                else list(range(9))
                            for di, d in enumerate(dlist):
                                dy, dx = OFFS[d]
                                z0 = 16 * ch + 2 - dy
                                x0 = 2 - dx
                                nc.tensor.matmul(
                                    prec[c][ch][:],
                                    bgT[t][:, 128 * (9 * c + d):
                                           128 * (9 * c + d + 1)],
                                    E[t][:, z0:z0 + 16, x0:x0 + 32],
                                    start=(t == 0 and di == 0),
                                    stop=(t == 7 and d == 8))
                        if t == 3:
                            # eps term: recovered += eps * ones_c (x) box(s2)
                            for ch in range(2):
                                nc.tensor.matmul(
                                    prec[c][ch][:], onesrow16[:],
                                    boxs2_16[:, 16 * ch:16 * ch + 16, :],
                                    start=False, stop=False)
                    if t == 7:
                        for c in range(2):
                            for ch in range(2):
                                r0 = 16 * ch
                                nc.vector.tensor_tensor(
                                    fscr[c][:, r0:r0 + 16, :], prec[c][ch][:],
                                    maskb9[:, r0:r0 + 16, :], ALU.mult)
                            nc.vector.tensor_tensor(
                                final_pad[c][:, 8:40, 8:40],
                                fscr[c][:], fscr2[c][:], ALU.add)

                if dbg:
                    ftmp = late.tile([128, 32, 32], F32, name="ftmp")
                    nc.scalar.copy(ftmp[:], final_pad[0][:, 8:40, 8:40])
                    nc.gpsimd.dma_start(dbg["d_final"][:], ftmp[:])

                # ---- dilated convs; evict/DMA per 64-channel half ----
                out_sb = [late.tile([128, 16, 32], F32, name=f"out_sb{i}",
                                    tag="osb", bufs=2) for i in range(4)]
                outq = [nc.sync, nc.scalar, nc.gpsimd, nc.sync,
                        nc.scalar, nc.gpsimd, nc.sync, nc.scalar]
                qi = 0
                for ct in range(2):
                    for ch in range(2):
                        pso = ps_o.tile([128, 512], F32, name="pso", tag="pso")
                        osb = out_sb[2 * ct + ch]

                        def half_mms(half, c):
                            g = 2 * ct + half
                            r = RATES[g]
                            for di, (dy, dx) in enumerate(OFFS):
                                oy = 8 + r * (dy - 1) + 16 * ch
                                ox = 8 + r * (dx - 1)
                                woff = 576 * g + 64 * (3 * dy + dx)
                                nc.tensor.matmul(
                                    pso[64 * half:64 * half + 64, :],
                                    wsb[c][:, woff:woff + 64],
                                    final_pad[c][:, oy:oy + 16, ox:ox + 32],
                                    start=(c == 0 and di == 0),
                                    stop=(c == 1 and di == 8),
                                    tile_position=(0, 64 * half))

                        def evict_half(half):
                            nonlocal qi
                            h0 = 64 * half
                            nc.scalar.activation(
                                osb[h0:h0 + 64, :],
                                pso[h0:h0 + 64, :]
                                .rearrange("p (a b) -> p a b", b=32),
                                AF.Relu, bias=biasb[ct][h0:h0 + 64, :])
                            outq[qi].dma_start(
                                out_d[128 * ct + h0:128 * ct + h0 + 64,
                                      16 * ch:16 * ch + 16, :],
                                osb[h0:h0 + 64, :])
                            qi += 1

                        half_mms(0, 0)
                        half_mms(1, 0)
                        half_mms(0, 1)
                        evict_half(0)
                        half_mms(1, 1)
                        evict_half(1)


def _get_nc():
    if "nc" not in _CACHE:
        _CACHE["nc"] = build_program()
    return _CACHE["nc"]


def _host_bgt(bgm16_pad, rn):
    """Transposed normalized patch bank [8,128,2304] fp16 (im2col of bgm)."""
    bgm = bgm16_pad.astype(np.float32)
    out = np.empty((1024, 2304), np.float32)
    for c in range(2):
        for d, (dy, dx) in enumerate(OFFS):
            blk = bgm[128 * c:128 * (c + 1), dy:dy + 32, dx:dx + 32]
            out[:, 128 * (9 * c + d):128 * (9 * c + d + 1)] = \
                blk.reshape(128, 1024).T
    out *= rn[:, None]
    return np.ascontiguousarray(out.astype(np.float16).reshape(8, 128, 2304))


def _host_aux_rows(fg, bg, mask):
    """Per-sample [4,1024] f32 rows: mask/9, 1-mask, eps*G, 1/norm."""
    m = mask.reshape(32, 32).astype(np.float64)
    mflat = m.reshape(1, -1)
    bgm = bg.astype(np.float64) * (1.0 - m)[None]
    colsum_fg = fg.astype(np.float64).sum(0)
    colsum_sq = (bgm ** 2).sum(0)
    colsum_s1 = bgm.sum(0)

    def box(x):
        xp = np.pad(x, 1)
        out = np.zeros((32, 32))
        for dy in range(3):
            for dx in range(3):
                out += xp[dy:dy + 32, dx:dx + 32]
        return out

    G = box(colsum_fg)
    ssq = box(colsum_sq)
    s1 = box(colsum_s1)
    norm = np.sqrt(ssq + 2.0 * EPS * s1 + 2304.0 * EPS * EPS)
    rn = 1.0 / norm.reshape(-1)
    # rn packed so a single contiguous DMA yields rncol[p, u] = rn[128u+p]
    rn_packed = rn.reshape(8, 128).T.reshape(-1)
    rows = np.stack([
        mflat[0] / 9.0,
        1.0 - mflat[0],
        EPS * G.reshape(-1),
        rn_packed,
    ]).astype(np.float32)
    return np.ascontiguousarray(rows), rn.astype(np.float32)


def kernel(foreground, mask, background, conv_w, conv_b):
    nc = _get_nc()
    fg = np.ascontiguousarray(foreground, dtype=np.float32)
    bg = np.ascontiguousarray(background, dtype=np.float32)
    m32 = np.asarray(mask, dtype=np.float32).reshape(32, 32)
    fg16 = np.zeros((8, 256, 34, 34), np.float16)
    fg16[:, :, 1:33, 1:33] = fg.astype(np.float16)
    bgm16 = np.zeros((8, 256, 34, 34), np.float16)
    bgm16[:, :, 1:33, 1:33] = (bg * (1.0 - m32)[None, None]).astype(np.float16)
    # conv_w [4,64,256,3,3] -> [c, g, dy, dx, o] -> [2, 128, 2304] fp16
    wre = np.ascontiguousarray(
        conv_w.astype(np.float32).transpose(2, 0, 3, 4, 1)
        .reshape(2, 128, 2304).astype(np.float16))
    bias = np.ascontiguousarray(conv_b.astype(np.float32).reshape(256, 1))
    in_maps = []
    for i in range(8):
        aux, rn = _host_aux_rows(fg[i], bg[i], mask)
        in_maps.append(
            {"fg16": fg16[i], "bgm16": bgm16[i], "aux": aux,
             "bgt": _host_bgt(bgm16[i], rn), "wconv": wre, "bias": bias})
    res = run_bass_kernel_spmd(nc, in_maps, list(range(8)))
    return np.stack([res.results[i]["out"] for i in range(8)], axis=0)


if __name__ == "__main__":
    build_program()
    print("build ok")


# revision 31
# speedup vs baseline: 1.0095x; 1.0095x over previous
"""Trainium2 Bass kernel for ContextualAttentionModule.

Data-parallel over batch: 8 samples -> 8 NeuronCores, one sample per core.
Per-core pipeline (C=256, H=W=32, L=1024 patches), v3:
  scores  = <fg_patch(p), bg_patch(l)> fp16 matmuls; +eps*G folded into
            psum eviction; /norm folded into exp scale
  prop    = 3x3 window-sum (separable DVE adds, f32)
  attn    = softmax over l (denominator via ones/rn column matmuls)
  recov   = conv_transpose(attn, kernels) via fp16 PE-transposed bank
  final   = recov*mask/9 + fg*(1-mask)
  out     = concat_g relu(dilated_conv_r(final) + b)  (fp16 matmuls)

Host ships small per-sample rows (mask/9, 1-mask, eps*G, 1/norm) so the
device spends no time on the scalar-row chains; all O(C*L*9) work (scores,
softmax, tconv, dilated convs) runs on device.

Engine rules honored: compute engines are lane-locked (partition i in ->
partition i out; SBUF operands of one op share a partition base; bases are
multiples of 32). Only PE, DMA and gpsimd partition_broadcast cross
partitions. GPSIMD cannot access PSUM.
"""

import numpy as np

import concourse.bass as bass
import concourse.tile as tile
from concourse import bacc, mybir
from concourse.bass_utils import run_bass_kernel_spmd
from concourse.masks import make_identity

F32 = mybir.dt.float32
F32R = mybir.dt.float32r
BF16 = mybir.dt.bfloat16
FP16 = mybir.dt.float16
U16 = mybir.dt.uint16
AF = mybir.ActivationFunctionType
ALU = mybir.AluOpType

EPS = 1e-7
RATES = (1, 2, 4, 8)
OFFS = [(dy, dx) for dy in range(3) for dx in range(3)]

_CACHE = {}


def build_program(debug=False):
    nc = bacc.Bacc()
    fg_d = nc.declare_dram_parameter("fg16", [256, 34, 34], FP16, isOutput=False)
    bg_d = nc.declare_dram_parameter("bgm16", [256, 34, 34], FP16, isOutput=False)
    m_d = nc.declare_dram_parameter("aux", [4, 1024], F32, isOutput=False)
    w_d = nc.declare_dram_parameter("wconv", [2, 128, 2304], FP16, isOutput=False)
    bgt_d = nc.declare_dram_parameter("bgt", [8, 128, 2304], FP16, isOutput=False)
    b_d = nc.declare_dram_parameter("bias", [256, 1], F32, isOutput=False)
    out_d = nc.declare_dram_parameter("out", [256, 32, 32], F32, isOutput=True)
    dbg = {}
    if debug:
        for nm, shp in [("d_drow", [1, 1024]), ("d_boxs2", [1, 1024]),
                        ("d_final", [128, 32, 32])]:
            dbg[nm] = nc.declare_dram_parameter(nm, shp, F32, isOutput=True)

    with tile.TileContext(nc) as tc:
        _emit(nc, tc, fg_d, bg_d, m_d, w_d, b_d, bgt_d, out_d, dbg)
    nc.compile()
    return nc


def _ring_zero16(nc, buf, eng, n=34):
    eng.memset(buf[:, 0:n:n - 1, :].bitcast(U16), 0)
    eng.memset(buf[:, 1:n - 1, 0:n:n - 1].bitcast(U16), 0)


def _ring_zero32(nc, buf, eng, n=34):
    eng.memset(buf[:, 0:n:n - 1, :].bitcast(F32), 0.0)
    eng.memset(buf[:, 1:n - 1, 0:n:n - 1].bitcast(F32), 0.0)


def _boxsum(nc, scr, src_pad, dst, eng):
    """3x3 SAME window sum on [p,34,34] ring-zeroed tiles -> [p,32,32]."""
    eng.tensor_tensor(scr[:, 1:33, 1:33], src_pad[:, 1:33, 0:32],
                      src_pad[:, 1:33, 1:33], ALU.add)
    eng.tensor_tensor(scr[:, 1:33, 1:33], scr[:, 1:33, 1:33],
                      src_pad[:, 1:33, 2:34], ALU.add)
    eng.tensor_tensor(dst[:], scr[:, 0:32, 1:33], scr[:, 1:33, 1:33], ALU.add)
    eng.tensor_tensor(dst[:], dst[:], scr[:, 2:34, 1:33], ALU.add)


def _emit(nc, tc, fg_d, bg_d, m_d, w_d, b_d, bgt_d, out_d, dbg=None):
    dbg = dbg or {}
    with tc.tile_pool(name="main", bufs=1) as main:
        # ----- persistent tiles -----
        fg_pad = [main.tile([128, 34, 34], FP16, name=f"fg_pad{c}") for c in range(2)]
        bgs = [[main.tile([128, 32, 32], FP16, name=f"bgs{c}_{d}") for d in range(9)]
               for c in range(2)]
        E = [main.tile([128, 34, 34], BF16, name=f"E{t}") for t in range(8)]
        bgT = [main.tile([128, 2304], FP16, name=f"bgT{t}") for t in range(8)]
        S = [main.tile([128, 34, 34], F32R, name=f"S{i}") for i in range(2)]
        WS = [main.tile([128, 34, 34], F32R, name=f"WS{i}") for i in range(2)]
        HS = [main.tile([128, 32, 32], F32R, name=f"HS{i}") for i in range(2)]
        maskb9 = main.tile([128, 32, 32], F32, name="maskb9")
        invmaskb = main.tile([128, 32, 32], F32, name="invmaskb")
        epsboxgB = main.tile([128, 32, 32], F32R, name="epsboxgB")
        Db = main.tile([128, 32, 32], F32R, name="Db")
        rncol = main.tile([128, 8], F32, name="rncol")
        W2 = main.tile([128, 16], BF16, name="W2")
        onesrow16 = main.tile([1, 128], FP16, name="onesrow16")
        wsb = [main.tile([128, 2304], FP16, name=f"wsb{c}") for c in range(2)]
        biasb = [main.tile([128, 1], F32, name=f"biasb{c}") for c in range(2)]
        fscr2 = [main.tile([128, 32, 32], FP16, name=f"fscr2_{c}") for c in range(2)]
        # Row scratch at partition 0 (s2 chain only).
        chainpad = main.tile([1, 34, 68], F32R, name="chainpad")
        cpad = chainpad[:, :, 0:34]
        cscr = chainpad[:, :, 34:68]
        boxA = main.tile([1, 32, 32], F32R, name="boxA")
        rowX = main.tile([1, 1024], F32R, name="rowX")   # rdrow (1/D)
        rowY = main.tile([1, 1024], F32, name="rowY")    # mask/9 row
        rowZ = main.tile([1, 1024], F32, name="rowZ")    # 1-mask row -> s2row
        rowG = main.tile([1, 1024], F32R, name="rowG")   # eps*G row
        s2stage = main.tile([33, 1024], F32, name="s2stage")
        boxs2_16 = main.tile([1, 32, 32], FP16, name="boxs2_16")

        with tc.tile_pool(name="stage", bufs=1) as stage:
            bg_pad = [stage.tile([128, 34, 34], FP16, name=f"bg_pad{c}")
                      for c in range(2)]

            # ----- DMAs (issue cost: Pool 25ns, SP 565ns, Act 667ns) -----
            nc.gpsimd.dma_start(rowY[0:1, :], m_d[0:1])
            nc.gpsimd.dma_start(rowZ[0:1, :], m_d[1:2])
            nc.gpsimd.dma_start(rowG[0:1, :].bitcast(F32), m_d[2:3])
            nc.sync.dma_start(bg_pad[0][:], bg_d[0:128])
            nc.sync.dma_start(bg_pad[1][:], bg_d[128:256])
            nc.scalar.dma_start(fg_pad[0][:], fg_d[0:128])
            nc.scalar.dma_start(fg_pad[1][:], fg_d[128:256])
            nc.sync.dma_start(rncol[:, :],
                              m_d[3:4].rearrange("o (p u) -> (o p) u", u=8))
            nc.sync.dma_start(wsb[0][:], w_d[0])
            nc.sync.dma_start(wsb[1][:], w_d[1])
            nc.sync.dma_start(biasb[0][:], b_d[0:128])
            nc.sync.dma_start(biasb[1][:], b_d[128:256])
            for t in range(8):
                (nc.sync if t % 2 == 0 else nc.scalar).dma_start(
                    bgT[t][:], bgt_d[t])

            # ----- Pool constants -----
            nc.gpsimd.memset(onesrow16[:], 1.0)
            nc.gpsimd.memset(W2[:], 1.0)

            # bgs shifts in psc consumption order:
            # DVE d{0,3,6}+c0d8, Act d{1,4,7}+c1d8, Pool d{2,5}
            # DVE is the fastest copier (2x ports): it takes the tiles the
            # first score matmul consumes earliest, Act/Pool take the rest
            for d in (0, 1, 3, 6, 8):
                dy, dx = OFFS[d]
                nc.vector.tensor_copy(bgs[0][d][:],
                                      bg_pad[0][:, dy:dy + 32, dx:dx + 32])
            for d in (0, 3, 6, 8):
                dy, dx = OFFS[d]
                nc.vector.tensor_copy(bgs[1][d][:],
                                      bg_pad[1][:, dy:dy + 32, dx:dx + 32])
            for d in (4, 7):
                dy, dx = OFFS[d]
                nc.scalar.copy(bgs[0][d][:],
                               bg_pad[0][:, dy:dy + 32, dx:dx + 32])
            for d in (1, 4, 7):
                dy, dx = OFFS[d]
                nc.scalar.copy(bgs[1][d][:],
                               bg_pad[1][:, dy:dy + 32, dx:dx + 32])
            for c in range(2):
                for d in (2, 5):
                    dy, dx = OFFS[d]
                    nc.gpsimd.tensor_copy(bgs[c][d][:],
                                          bg_pad[c][:, dy:dy + 32, dx:dx + 32])
            nc.gpsimd.partition_broadcast(
                epsboxgB.rearrange("p a b -> p (a b)"), rowG[0:1, :])
            nc.gpsimd.partition_broadcast(
                invmaskb.rearrange("p a b -> p (a b)"), rowZ[0:1, :])
            nc.gpsimd.partition_broadcast(
                maskb9.rearrange("p a b -> p (a b)"), rowY[0:1, :])
            _ring_zero32(nc, cpad, nc.vector)
            _ring_zero32(nc, cscr, nc.vector)
            for i in range(2):
                _ring_zero32(nc, S[i], nc.vector)
                _ring_zero32(nc, WS[i], nc.vector)
            for t in range(8):
                _ring_zero16(nc, E[t], nc.gpsimd)
            # fg*(1-mask), staged for phase 7 (Pool has slack here)
            for c in range(2):
                nc.gpsimd.tensor_tensor(fscr2[c][:], fg_pad[c][:, 1:33, 1:33],
                                        invmaskb[:], ALU.mult)

            # ================= scores phase =================
            with (
                tc.tile_pool(name="ps_sc", bufs=4, space="PSUM") as ps_sc,
                tc.tile_pool(name="ps_d", bufs=2, space="PSUM") as ps_d,
            ):
                blocks = [(c, d) for c in range(2) for d in range(9)]

                def psc_mms(t, ch):
                    psc = ps_sc.tile([128, 512], F32, name="psc", tag="psc")
                    r0 = 16 * ch
                    i = 0
                    for c in range(2):
                        for d, (dy, dx) in enumerate(OFFS):
                            nc.tensor.matmul(
                                psc[:],
                                bgs[c][d].rearrange("p a b -> p (a b)")
                                [:, 128 * t:128 * (t + 1)],
                                fg_pad[c][:, r0 + dy:r0 + dy + 16, dx:dx + 32],
                                start=(i == 0), stop=(i == 17))
                            i += 1
                    return psc

                # psd[ch]: partition 0 = D = sum_l E; partition 32 = sum_l rn*E
                psd = [ps_d.tile([33, 512], F32, name=f"psd{ch}", tag="prd")
                       for ch in range(2)]

                def psd_mm(u, chs=(0, 1)):
                    for ch in chs:
                        r0 = 16 * ch
                        mv = E[u][:, 1 + r0:17 + r0, 1:33]
                        nc.tensor.matmul(psd[ch][0:1, :], W2[:, 2 * u:2 * u + 1],
                                         mv, start=(u == 0), stop=(u == 7))
                        nc.tensor.matmul(psd[ch][32:33, :],
                                         W2[:, 2 * u + 1:2 * u + 2],
                                         mv, start=(u == 0), stop=(u == 7))

                def boxexp(t):
                    w, h = WS[t % 2], HS[t % 2]
                    sp = S[t % 2]
                    nc.vector.tensor_tensor(w[:, 1:33, 1:33], sp[:, 1:33, 0:32],
                                            sp[:, 1:33, 1:33], ALU.add)
                    nc.vector.tensor_tensor(w[:, 1:33, 1:33], w[:, 1:33, 1:33],
                                            sp[:, 1:33, 2:34], ALU.add)
                    nc.vector.tensor_tensor(h[:], w[:, 0:32, 1:33],
                                            w[:, 1:33, 1:33], ALU.add)
                    nc.vector.tensor_tensor(h[:], h[:], w[:, 2:34, 1:33], ALU.add)
                    nc.scalar.activation(E[t][:, 1:33, 1:33], h[:], AF.Exp,
                                         scale=rncol[:, t:t + 1])

                def boxexp_split(t):
                    # final slot: split rows DVE/Pool and pipeline exp halves
                    # with the closing denominator matmuls
                    w, h = WS[t % 2], HS[t % 2]
                    sp = S[t % 2]
                    nc.vector.tensor_tensor(w[:, 1:18, 1:33], sp[:, 1:18, 0:32],
                                            sp[:, 1:18, 1:33], ALU.add)
                    nc.vector.tensor_tensor(w[:, 1:18, 1:33], w[:, 1:18, 1:33],
                                            sp[:, 1:18, 2:34], ALU.add)
                    nc.gpsimd.tensor_tensor(w[:, 18:33, 1:33], sp[:, 18:33, 0:32],
                                            sp[:, 18:33, 1:33], ALU.add)
                    nc.gpsimd.tensor_tensor(w[:, 18:33, 1:33], w[:, 18:33, 1:33],
                                            sp[:, 18:33, 2:34], ALU.add)
                    nc.vector.tensor_tensor(h[:, 0:16, :], w[:, 0:16, 1:33],
                                            w[:, 1:17, 1:33], ALU.add)
                    nc.vector.tensor_tensor(h[:, 0:16, :], h[:, 0:16, :],
                                            w[:, 2:18, 1:33], ALU.add)
                    nc.scalar.activation(E[t][:, 1:17, 1:33], h[:, 0:16, :],
                                         AF.Exp, scale=rncol[:, t:t + 1])
                    psd_mm(t, chs=(0,))
                    nc.vector.tensor_tensor(h[:, 16:32, :], w[:, 16:32, 1:33],
                                            w[:, 17:33, 1:33], ALU.add)
                    nc.vector.tensor_tensor(h[:, 16:32, :], h[:, 16:32, :],
                                            w[:, 18:34, 1:33], ALU.add)
                    nc.scalar.activation(E[t][:, 17:33, 1:33], h[:, 16:32, :],
                                         AF.Exp, scale=rncol[:, t:t + 1])
                    psd_mm(t, chs=(1,))

                def evict(t, ch, psc):
                    r0 = 16 * ch
                    nc.vector.tensor_tensor(
                        S[t % 2][:, 1 + r0:17 + r0, 1:33], psc[:],
                        epsboxgB[:, r0:16 + r0, :], ALU.add)

                # ---- slots 0..6 ----
                for t in range(7):
                    for ch in range(2):
                        psc = psc_mms(t, ch)
                        evict(t, ch, psc)
                    boxexp(t)
                    if t == 0:
                        # W2 = [1, rn_0, 1, rn_1, ...] (bf16)
                        nc.vector.tensor_copy(W2[:, 1:16:2], rncol[:, 0:8])
                    # PE tail of slot: lagged denominator matmuls
                    if t >= 2:
                        psd_mm(t - 2)
                # ---- slot 7: close out denominators while E7 resolves ----
                for ch in range(2):
                    psc = psc_mms(7, ch)
                    evict(7, ch, psc)
                psd_mm(5)
                psd_mm(6)
                # boxexp_split emits psd(7) halves interleaved with exp
                boxexp_split(7)

                # denominator -> reciprocal -> broadcast -> divide; E[0]
                # is divided per 16-row half so tconv can start earlier
                for ch in range(2):
                    nc.vector.reciprocal(
                        rowX[0:1, 512 * ch:512 * (ch + 1)].bitcast(F32),
                        psd[ch][0:1, :])
                    nc.gpsimd.partition_broadcast(
                        Db[:, 16 * ch:16 * (ch + 1), :]
                        .rearrange("p a b -> p (a b)"),
                        rowX[0:1, 512 * ch:512 * (ch + 1)])
                    r0 = 16 * ch
                    nc.vector.tensor_tensor(E[0][:, 1 + r0:17 + r0, 1:33],
                                            E[0][:, 1 + r0:17 + r0, 1:33],
                                            Db[:, r0:16 + r0, :], ALU.mult)
                for t in range(1, 8):
                    if t < 5:
                        nc.vector.tensor_tensor(E[t][:, 1:33, 1:33],
                                                E[t][:, 1:33, 1:33], Db[:],
                                                ALU.mult)
                    else:
                        nc.gpsimd.tensor_tensor(E[t][:, 1:33, 1:33],
                                                E[t][:, 1:33, 1:33], Db[:],
                                                ALU.mult)
                # s2 = eps * (sum_l rn*E) / D; lane-aligned psum escape via
                # Act copy (32->32) then cross-partition DMA (32->0)
                for ch in range(2):
                    nc.scalar.copy(s2stage[32:33, 512 * ch:512 * (ch + 1)],
                                   psd[ch][32:33, :])
                nc.sync.dma_start(rowZ[0:1, 0:512], s2stage[32:33, 0:512])
                nc.gpsimd.dma_start(rowZ[0:1, 512:1024],
                                    s2stage[32:33, 512:1024])
                for ch in range(2):
                    r0 = 16 * ch
                    nc.vector.scalar_tensor_tensor(
                        out=cpad[:, 1 + r0:17 + r0, 1:33],
                        in0=rowZ[0:1, 512 * ch:512 * (ch + 1)], scalar=EPS,
                        in1=rowX[0:1, 512 * ch:512 * (ch + 1)],
                        op0=ALU.mult, op1=ALU.mult)
                _boxsum(nc, cscr, cpad, boxA, nc.vector)
                nc.vector.tensor_copy(boxs2_16[:], boxA)
                if dbg:
                    nc.sync.dma_start(dbg["d_drow"][:], rowX[0:1, :].bitcast(F32))
                    nc.sync.dma_start(
                        dbg["d_boxs2"][:],
                        boxA.rearrange("o a b -> o (a b)").bitcast(F32))
        # ----- stage + scores psum pools closed -----

        with tc.tile_pool(name="late", bufs=1) as late:
            final_pad = [late.tile([128, 48, 48], FP16, name=f"final_pad{c}")
                         for c in range(2)]
            fscr = [late.tile([128, 32, 32], F32, name=f"fscr{c}") for c in range(2)]
            for c in range(2):
                nc.gpsimd.memset(final_pad[c][:, 0:8, :].bitcast(U16), 0)
                nc.gpsimd.memset(final_pad[c][:, 40:48, :].bitcast(U16), 0)
                nc.gpsimd.memset(final_pad[c][:, 8:40, 0:8].bitcast(U16), 0)
                nc.gpsimd.memset(final_pad[c][:, 8:40, 40:48].bitcast(U16), 0)

            with (
                tc.tile_pool(name="ps_rec", bufs=1, space="PSUM") as ps_rec,
                tc.tile_pool(name="ps_o", bufs=3, space="PSUM") as ps_o,
            ):
                prec = [[ps_rec.tile([128, 512], F32, name=f"prec{c}_{ch}")
                         for ch in range(2)] for c in range(2)]
                # ---- tconv: contraction over (l, d), c-outer ----
                d_first = [6, 7, 8, 3, 4, 5, 0, 1, 2]  # dy=2 rows first
                for t in range(8):
                    for c in range(2):
                        for ch in range(2):
                            dlist = d_first if (c == 0 and t == 0) \
                                else list(range(9))
                            for di, d in enumerate(dlist):
                                dy, dx = OFFS[d]
                                z0 = 16 * ch + 2 - dy
                                x0 = 2 - dx
                                nc.tensor.matmul(
                                    prec[c][ch][:],
                                    bgT[t][:, 128 * (9 * c + d):
                                           128 * (9 * c + d + 1)],
                                    E[t][:, z0:z0 + 16, x0:x0 + 32],
                                    start=(t == 0 and di == 0),
                                    stop=(t == 7 and d == 8))
                        if t == 3:
                            # eps term: recovered += eps * ones_c (x) box(s2)
                            for ch in range(2):
                                nc.tensor.matmul(
                                    prec[c][ch][:], onesrow16[:],
                                    boxs2_16[:, 16 * ch:16 * ch + 16, :],
                                    start=False, stop=False)
                    if t == 7:
                        for c in range(2):
                            for ch in range(2):
                                r0 = 16 * ch
                                nc.vector.tensor_tensor(
                                    fscr[c][:, r0:r0 + 16, :], prec[c][ch][:],
                                    maskb9[:, r0:r0 + 16, :], ALU.mult)
                            nc.vector.tensor_tensor(
                                final_pad[c][:, 8:40, 8:40],
                                fscr[c][:], fscr2[c][:], ALU.add)

                if dbg:
                    ftmp = late.tile([128, 32, 32], F32, name="ftmp")
                    nc.scalar.copy(ftmp[:], final_pad[0][:, 8:40, 8:40])
                    nc.gpsimd.dma_start(dbg["d_final"][:], ftmp[:])

                # ---- dilated convs; evict/DMA per 64-channel half ----
                out_sb = [late.tile([128, 16, 32], F32, name=f"out_sb{i}",
                                    tag="osb", bufs=2) for i in range(4)]
                outq = [nc.sync, nc.scalar, nc.gpsimd, nc.sync,
                        nc.scalar, nc.gpsimd, nc.sync, nc.scalar]
                qi = 0
                for ct in range(2):
                    for ch in range(2):
                        pso = ps_o.tile([128, 512], F32, name="pso", tag="pso")
                        osb = out_sb[2 * ct + ch]

                        def half_mms(half, c):
                            g = 2 * ct + half
                            r = RATES[g]
                            for di, (dy, dx) in enumerate(OFFS):
                                oy = 8 + r * (dy - 1) + 16 * ch
                                ox = 8 + r * (dx - 1)
                                woff = 576 * g + 64 * (3 * dy + dx)
                                nc.tensor.matmul(
                                    pso[64 * half:64 * half + 64, :],
                                    wsb[c][:, woff:woff + 64],
                                    final_pad[c][:, oy:oy + 16, ox:ox + 32],
                                    start=(c == 0 and di == 0),
                                    stop=(c == 1 and di == 8),
                                    tile_position=(0, 64 * half))

                        def evict_half(half):
                            nonlocal qi
                            h0 = 64 * half
                            nc.scalar.activation(
                                osb[h0:h0 + 64, :],
                                pso[h0:h0 + 64, :]
                                .rearrange("p (a b) -> p a b", b=32),
                                AF.Relu, bias=biasb[ct][h0:h0 + 64, :])
                            outq[qi].dma_start(
                                out_d[128 * ct + h0:128 * ct + h0 + 64,
                                      16 * ch:16 * ch + 16, :],
                                osb[h0:h0 + 64, :])
                            qi += 1

                        half_mms(0, 0)
                        half_mms(1, 0)
                        half_mms(0, 1)
                        evict_half(0)
                        half_mms(1, 1)
                        evict_half(1)


def _get_nc():
    if "nc" not in _CACHE:
        _CACHE["nc"] = build_program()
    return _CACHE["nc"]


def _host_bgt(bgm16_pad, rn):
    """Transposed normalized patch bank [8,128,2304] fp16 (im2col of bgm)."""
    bgm = bgm16_pad.astype(np.float32)
    out = np.empty((1024, 2304), np.float32)
    for c in range(2):
        for d, (dy, dx) in enumerate(OFFS):
            blk = bgm[128 * c:128 * (c + 1), dy:dy + 32, dx:dx + 32]
            out[:, 128 * (9 * c + d):128 * (9 * c + d + 1)] = \
                blk.reshape(128, 1024).T
    out *= rn[:, None]
    return np.ascontiguousarray(out.astype(np.float16).reshape(8, 128, 2304))


def _host_aux_rows(fg, bg, mask):
    """Per-sample [4,1024] f32 rows: mask/9, 1-mask, eps*G, 1/norm."""
    m = mask.reshape(32, 32).astype(np.float64)
    mflat = m.reshape(1, -1)
    bgm = bg.astype(np.float64) * (1.0 - m)[None]
    colsum_fg = fg.astype(np.float64).sum(0)
    colsum_sq = (bgm ** 2).sum(0)
    colsum_s1 = bgm.sum(0)

    def box(x):
        xp = np.pad(x, 1)
        out = np.zeros((32, 32))
        for dy in range(3):
            for dx in range(3):
                out += xp[dy:dy + 32, dx:dx + 32]
        return out

    G = box(colsum_fg)
    ssq = box(colsum_sq)
    s1 = box(colsum_s1)
    norm = np.sqrt(ssq + 2.0 * EPS * s1 + 2304.0 * EPS * EPS)
    rn = 1.0 / norm.reshape(-1)
    # rn packed so a single contiguous DMA yields rncol[p, u] = rn[128u+p]
    rn_packed = rn.reshape(8, 128).T.reshape(-1)
    rows = np.stack([
        mflat[0] / 9.0,
        1.0 - mflat[0],
        EPS * G.reshape(-1),
        rn_packed,
    ]).astype(np.float32)
    return np.ascontiguousarray(rows), rn.astype(np.float32)


def kernel(foreground, mask, background, conv_w, conv_b):
    nc = _get_nc()
    fg = np.ascontiguousarray(foreground, dtype=np.float32)
    bg = np.ascontiguousarray(background, dtype=np.float32)
    m32 = np.asarray(mask, dtype=np.float32).reshape(32, 32)
    fg16 = np.zeros((8, 256, 34, 34), np.float16)
    fg16[:, :, 1:33, 1:33] = fg.astype(np.float16)
    bgm16 = np.zeros((8, 256, 34, 34), np.float16)
    bgm16[:, :, 1:33, 1:33] = (bg * (1.0 - m32)[None, None]).astype(np.float16)
    # conv_w [4,64,256,3,3] -> [c, g, dy, dx, o] -> [2, 128, 2304] fp16
    wre = np.ascontiguousarray(
        conv_w.astype(np.float32).transpose(2, 0, 3, 4, 1)
        .reshape(2, 128, 2304).astype(np.float16))
    bias = np.ascontiguousarray(conv_b.astype(np.float32).reshape(256, 1))
    in_maps = []
    for i in range(8):
        aux, rn = _host_aux_rows(fg[i], bg[i], mask)
        in_maps.append(
            {"fg16": fg16[i], "bgm16": bgm16[i], "aux": aux,
             "bgt": _host_bgt(bgm16[i], rn), "wconv": wre, "bias": bias})
    res = run_bass_kernel_spmd(nc, in_maps, list(range(8)))
    return np.stack([res.results[i]["out"] for i in range(8)], axis=0)


if __name__ == "__main__":
    build_program()
    print("build ok")
